# revision 1
# baseline (speedup 1.0000x reference)
"""Multi-head causal attention (B=2, S=2048, D=4096, H=32, hd=128) on 8 trn2 cores.

Sharding: DP over batch (2) x TP over heads (4 groups of 8 heads).
Core c: batch b = c//4, head-group tp = c%4.
Each core computes a partial output [2048, 4096] (wo row-sharded); host sums
the 4 partials per batch.

All matmuls run as float32r (full PE rate, ~tf32 precision).
Host pre-transposes x / weights / mask so every DMA is natural-layout.
q/k head dims are de-interleaved (evens then odds) on the host so RoPE becomes
full-tile DVE ops on partition halves; the permutation is consistent between
q and k so scores are unchanged. v / wo stay in natural order.
Scores are computed transposed ([tk, tq]) so the PV matmul needs no
on-chip transpose of the probabilities; softmax is unnormalized exp with the
denominator from a ones-vector matmul, divided into the attention output.
"""

import sys
sys.path.insert(0, '/opt/trn_rl_repo')
sys.path.insert(0, '/opt/trn_rl_repo/concourse')

import numpy as np

S = 2048
D = 4096
HD = 128
FSH = 1024            # features per core (8 heads)
NHL = 8               # heads per core
KT = D // 128         # 32 k-tiles for projections
TSTRIPS = S // 512    # 4 tq strips
NKT = S // 128        # 16 tk tiles
NEG_THRESH = -1.0e8

_cache = {}


def _build(classes):
    """Build + compile the per-core Bacc program. classes[j][s] in {0:skip,1:zero,2:add}."""
    import concourse.bacc as bacc
    import concourse.mybir as mybir
    import concourse.tile as tile

    f32 = mybir.dt.float32
    f32r = mybir.dt.float32r
    EXP = mybir.ActivationFunctionType.Exp

    nc = bacc.Bacc("TRN2", target_bir_lowering=False, debug=False)

    xt_d = nc.dram_tensor("xt", [D, S], f32r, kind="ExternalInput").ap()
    wqt_d = nc.dram_tensor("wqt", [D, FSH], f32r, kind="ExternalInput").ap()
    wkt_d = nc.dram_tensor("wkt", [D, FSH], f32r, kind="ExternalInput").ap()
    wvt_d = nc.dram_tensor("wvt", [D, FSH], f32r, kind="ExternalInput").ap()
    wot_d = nc.dram_tensor("wot", [FSH, D], f32r, kind="ExternalInput").ap()
    cos_d = nc.dram_tensor("cosw", [64, S], f32, kind="ExternalInput").ap()
    sin_d = nc.dram_tensor("sinw", [64, S], f32, kind="ExternalInput").ap()
    nsin_d = nc.dram_tensor("nsinw", [64, S], f32, kind="ExternalInput").ap()
    mask_d = nc.dram_tensor("maskt", [S, S], f32, kind="ExternalInput").ap()
    id_d = nc.dram_tensor("id128", [128, 128], f32r, kind="ExternalInput").ap()
    on_d = nc.dram_tensor("ones128", [128, 128], f32r, kind="ExternalInput").ap()
    out_d = nc.dram_tensor("out", [S, D], f32, kind="ExternalOutput").ap()

    with tile.TileContext(nc) as tc, \
         nc.allow_low_precision(reason="float32r is 4-byte near-fp32"):
        with tc.tile_pool(name="pdram", bufs=1, space="DRAM") as pdram, \
             tc.tile_pool(name="pconst", bufs=1) as pconst:
            qt_d = pdram.tile([FSH, S], f32r, name="qt_spill")
            kt_d = pdram.tile([FSH, S], f32r, name="kt_spill")
            vt_d = pdram.tile([FSH, S], f32r, name="vt_spill")
            ones_sb = pconst.tile([128, 128], f32r, name="ones_sb")
            nc.sync.dma_start(out=ones_sb, in_=on_d)
            id_sb = pconst.tile([128, 128], f32r, name="id_sb")
            nc.sync.dma_start(out=id_sb, in_=id_d)
            ones_k = ones_sb[:, 0:1]
            ones_c = ones_sb[0:1, :]

            # ---------------- Phase 1: q/k/v projections (+RoPE on q,k) -------------
            with tc.tile_pool(name="p1x", bufs=KT) as p1x, \
                 tc.tile_pool(name="p1w", bufs=2) as p1w, \
                 tc.tile_pool(name="p1t", bufs=4) as p1t, \
                 tc.tile_pool(name="p1o", bufs=4) as p1o, \
                 tc.tile_pool(name="p1cs", bufs=1) as p1cs, \
                 tc.tile_pool(name="ps1", bufs=4, space="PSUM") as ps1:
                w_ds = [wqt_d, wkt_d, wvt_d]
                spills = [qt_d, kt_d, vt_d]
                jobs = [(proj, i) for proj in range(3) for i in range(NHL)]

                def load_w(proj, i):
                    wt = p1w.tile([128, KT, 128], f32r, name="wt")
                    w_ap = w_ds[proj][:, i * 128:(i + 1) * 128].rearrange(
                        "(k p) f -> p k f", p=128)
                    nc.sync.dma_start(out=wt, in_=w_ap)
                    return wt

                for T2 in range(2):           # t-strips of 1024
                    t0 = T2 * 1024
                    wt_next = load_w(*jobs[0])
                    xk = []
                    for k in range(KT):
                        xt_t = p1x.tile([128, 1024], f32r, name="xk")
                        nc.sync.dma_start(out=xt_t, in_=xt_d[k * 128:(k + 1) * 128, t0:t0 + 1024])
                        xk.append(xt_t)
                    cos_sb = p1cs.tile([64, 1024], f32, name="cos_sb")
                    sin_sb = p1cs.tile([64, 1024], f32, name="sin_sb")
                    nsin_sb = p1cs.tile([64, 1024], f32, name="nsin_sb")
                    nc.sync.dma_start(out=cos_sb, in_=cos_d[:, t0:t0 + 1024])
                    nc.sync.dma_start(out=sin_sb, in_=sin_d[:, t0:t0 + 1024])
                    nc.sync.dma_start(out=nsin_sb, in_=nsin_d[:, t0:t0 + 1024])

                    for idx, (proj, i) in enumerate(jobs):
                        wt = wt_next
                        if idx + 1 < len(jobs):
                            wt_next = load_w(*jobs[idx + 1])
                        spill = spills[proj]
                        if True:
                            for tsub in range(2):
                                ps = ps1.tile([128, 512], f32, name="ps1")
                                for k in range(KT):
                                    nc.tensor.matmul(
                                        ps, wt[:, k, :],
                                        xk[k][:, tsub * 512:(tsub + 1) * 512],
                                        start=(k == 0), stop=(k == KT - 1))
                                ot = p1o.tile([128, 512], f32r, name="ot")
                                csl = slice(tsub * 512, (tsub + 1) * 512)
                                if proj < 2:  # RoPE for q, k
                                    m1 = p1t.tile([64, 512], f32, name="m1")
                                    m2 = p1t.tile([64, 512], f32, name="m2")
                                    nc.vector.tensor_mul(m1, ps[0:64], cos_sb[:, csl])
                                    nc.vector.tensor_mul(m2, ps[64:128], nsin_sb[:, csl])
                                    nc.vector.tensor_add(ot[0:64], m1, m2)
                                    m3 = p1t.tile([64, 512], f32, name="m1")
                                    m4 = p1t.tile([64, 512], f32, name="m2")
                                    nc.vector.tensor_mul(m3, ps[0:64], sin_sb[:, csl])
                                    nc.vector.tensor_mul(m4, ps[64:128], cos_sb[:, csl])
                                    nc.vector.tensor_add(ot[64:128], m3, m4)
                                else:
                                    nc.vector.tensor_copy(ot, ps)
                                nc.gpsimd.dma_start(
                                    out=spill[i * 128:(i + 1) * 128,
                                              t0 + tsub * 512:t0 + (tsub + 1) * 512],
                                    in_=ot)


            # ---------------- Phase 2: attention per head ----------------------------
            with tc.tile_pool(name="patt", bufs=1) as patt, \
                 tc.tile_pool(name="p3w", bufs=2) as p3w:
              att_sb = [patt.tile([128, S], f32r, name=f"attT{h}") for h in range(NHL)]
              with tc.tile_pool(name="p2h", bufs=2) as p2h, \
                   tc.tile_pool(name="p2v", bufs=2 * NKT + 1) as p2v, \
                   tc.tile_pool(name="p2e", bufs=6) as p2e, \
                   tc.tile_pool(name="p2mt", bufs=6) as p2mt, \
                   tc.tile_pool(name="p2ms", bufs=3) as p2ms, \
                   tc.tile_pool(name="p2r", bufs=4) as p2r, \
                   tc.tile_pool(name="p2o", bufs=4) as p2o, \
                   tc.tile_pool(name="ps2s", bufs=3, space="PSUM") as ps2s, \
                   tc.tile_pool(name="ps2a", bufs=2, space="PSUM") as ps2a, \
                   tc.tile_pool(name="ps2t", bufs=2, space="PSUM") as ps2t, \
                   tc.tile_pool(name="ps2d", bufs=1, space="PSUM") as ps2d:
                  for h in range(NHL):
                      vt_h = p2h.tile([128, S], f32r, name="vt_h")
                      kt_h = p2h.tile([128, S], f32r, name="kt_h")
                      qt_h = p2h.tile([128, S], f32r, name="qt_h")
                      nc.sync.dma_start(out=vt_h, in_=vt_d[h * 128:(h + 1) * 128, :])
                      nc.sync.dma_start(out=kt_h, in_=kt_d[h * 128:(h + 1) * 128, :])
                      nc.sync.dma_start(out=qt_h, in_=qt_d[h * 128:(h + 1) * 128, :])
                      v_sb = []
                      for j in range(NKT):
                          tps = ps2t.tile([128, 128], f32r, name="tp")
                          nc.tensor.transpose(tps, vt_h[:, j * 128:(j + 1) * 128], id_sb)
                          vj = p2v.tile([128, 128], f32r, name="vj")
                          nc.vector.tensor_copy(vj, tps)
                          v_sb.append(vj)
                      for s in range(TSTRIPS):
                          act = [j for j in range(NKT) if classes[j][s] != 0]
                          A = ps2a.tile([128, 512], f32, name="A")
                          Dn = ps2d.tile([1, 512], f32, name="Dn")
                          qs = qt_h[:, s * 512:(s + 1) * 512]
                          for idx, j in enumerate(act):
                              sps = ps2s.tile([128, 512], f32, name="sps")
                              nc.tensor.matmul(sps, kt_h[:, j * 128:(j + 1) * 128],
                                               qs, start=True, stop=True)
                              E = p2e.tile([128, 512], f32r, name="E")
                              if classes[j][s] == 2:
                                  mt = p2mt.tile([128, 512], f32, name="mt")
                                  nc.sync.dma_start(
                                      out=mt,
                                      in_=mask_d[j * 128:(j + 1) * 128,
                                                 s * 512:(s + 1) * 512])
                                  ms = p2ms.tile([128, 512], f32, name="ms")
                                  nc.vector.tensor_add(ms, sps, mt)
                                  nc.scalar.activation(E, ms, EXP)
                              else:
                                  nc.scalar.activation(E, sps, EXP)
                              first, last = (idx == 0), (idx == len(act) - 1)
                              nc.tensor.matmul(A, v_sb[j], E, start=first, stop=last)
                              nc.tensor.matmul(Dn, ones_k, E, start=first, stop=last)
                          rec = p2r.tile([1, 512], f32r, name="rec")
                          nc.vector.reciprocal(rec, Dn[0:1, :])
                          bsb = p2o.tile([128, 512], f32r, name="bsb")
                          nc.gpsimd.partition_broadcast(bsb, rec, 128)
                          nc.vector.tensor_mul(
                              att_sb[h][:, s * 512:(s + 1) * 512], A, bsb)

              # ---------------- Phase 3: output projection ------------------------------
              if True:
                with tc.tile_pool(name="p3o", bufs=4) as p3o, \
                   tc.tile_pool(name="ps3", bufs=4, space="PSUM") as ps3:
                  def load_w3(c):
                      wt = p3w.tile([128, NHL, 512], f32r, name="w3")
                      w_ap = wot_d[:, c * 512:(c + 1) * 512].rearrange(
                          "(k p) f -> p k f", p=128)
                      nc.sync.dma_start(out=wt, in_=w_ap)
                      return wt

                  wt_next3 = load_w3(0)
                  am = att_sb
                  for c in range(8):        # dout chunks of 512
                      wt = wt_next3
                      if c + 1 < 8:
                          wt_next3 = load_w3(c + 1)
                      for m in range(NKT):  # t tiles of 128
                          ps = ps3.tile([128, 512], f32, name="ps3")
                          for k in range(NHL):
                              nc.tensor.matmul(ps, am[k][:, m * 128:(m + 1) * 128],
                                               wt[:, k, :],
                                               start=(k == 0), stop=(k == NHL - 1))
                          ot = p3o.tile([128, 512], f32, name="o3")
                          nc.vector.tensor_copy(ot, ps)
                          nc.gpsimd.dma_start(
                              out=out_d[m * 128:(m + 1) * 128, c * 512:(c + 1) * 512],
                              in_=ot)

    nc.compile()
    return nc


def _host_prep(x, wq, wk, wv, wo, freqs_cos, freqs_sin, mask):
    """Build per-core input maps + mask block classes."""
    x = np.asarray(x, np.float32)
    wq = np.asarray(wq, np.float32)
    wk = np.asarray(wk, np.float32)
    wv = np.asarray(wv, np.float32)
    wo = np.asarray(wo, np.float32)
    mask2 = np.asarray(mask, np.float32).reshape(S, S)

    perm = np.concatenate(
        [hl * 128 + np.concatenate([np.arange(0, 128, 2), np.arange(1, 128, 2)])
         for hl in range(NHL)])
    cosw = np.ascontiguousarray(np.asarray(freqs_cos, np.float32).T)
    sinw = np.ascontiguousarray(np.asarray(freqs_sin, np.float32).T)
    nsinw = np.ascontiguousarray(-sinw)
    maskt = np.ascontiguousarray(mask2.T)
    id128 = np.eye(128, dtype=np.float32)

    classes = [[0] * TSTRIPS for _ in range(NKT)]
    for j in range(NKT):
        for s in range(TSTRIPS):
            blk = maskt[j * 128:(j + 1) * 128, s * 512:(s + 1) * 512]
            if (blk <= NEG_THRESH).all():
                classes[j][s] = 0
            elif (blk == 0.0).all():
                classes[j][s] = 1
            else:
                classes[j][s] = 2

    xts = [np.ascontiguousarray(x[b].T) for b in range(2)]
    in_maps = []
    for core in range(8):
        b, tp = core // 4, core % 4
        sl = slice(tp * FSH, (tp + 1) * FSH)
        wq_c = wq[sl][perm] * np.float32(1.0 / np.sqrt(HD))
        wk_c = wk[sl][perm]
        in_maps.append({
            "xt": xts[b],
            "wqt": np.ascontiguousarray(wq_c.T),
            "wkt": np.ascontiguousarray(wk_c.T),
            "wvt": np.ascontiguousarray(wv[sl].T),
            "wot": np.ascontiguousarray(wo[:, sl].T),
            "cosw": cosw, "sinw": sinw, "nsinw": nsinw,
            "maskt": maskt, "id128": id128,
            "ones128": np.ones((128, 128), np.float32),
        })
    return in_maps, classes


def kernel(x, wq, wk, wv, wo, freqs_cos, freqs_sin, mask, start_pos=0,
           _trace=False):
    from concourse import bass_utils
    in_maps, classes = _host_prep(x, wq, wk, wv, wo, freqs_cos, freqs_sin, mask)
    key = str(classes)
    if key not in _cache:
        _cache[key] = _build(classes)
    nc = _cache[key]
    res = bass_utils.run_bass_kernel_spmd(nc, in_maps, core_ids=list(range(8)),
                                          trace=_trace)
    out = np.zeros((2, S, D), np.float32)
    for core in range(8):
        out[core // 4] += res.results[core]["out"]
    kernel.last_result = res
    return out


if __name__ == "__main__":
    # compile-only smoke test
    classes = [[2 if j * 128 <= s * 512 + 511 and j * 128 + 127 > s * 512 else
                (1 if j * 128 + 127 <= s * 512 else 0)
                for s in range(TSTRIPS)] for j in range(NKT)]
    import time
    t0 = time.time()
    nc = _build(classes)
    print(f"build+bacc-compile: {time.time()-t0:.1f}s")
    if len(sys.argv) > 1 and sys.argv[1] == "neff":
        import tempfile
        from concourse import bass_utils
        t0 = time.time()
        with tempfile.TemporaryDirectory() as td:
            bass_utils.compile_bass_kernel(nc, td)
            print(f"walrus: {time.time()-t0:.1f}s COMPILED OK")



# revision 4
# speedup vs baseline: 46.7509x; 46.7509x over previous
"""Multi-head causal attention (B=2, S=2048, D=4096, H=32, hd=128) on 8 trn2 cores.

Sharding: DP over batch (2) x TP over heads (4 groups of 8 heads).
Core c: batch b = c//4, head-group tp = c%4.
Each core computes a partial output [2048, 4096] (wo row-sharded); host sums
the 4 partials per batch.

v3 design (f16 datapath, SBUF-resident intermediates, single packed input):
- All matmul operands are float16 (PE runs f16 at 1 cycle/row, same as f32r,
  but DMA bytes and SBUF footprint halve; PSUM accumulation stays f32).
- All per-core inputs live in ONE packed DRAM tensor (the axon dispatch path
  costs ~70us/exec per IO buffer, independent of bytes) viewed through
  sliced/rearranged APs.
- q/k/v^T/att live entirely in SBUF between phases -- no DRAM spill.
- V^T is produced directly by the projection matmul (stationary = x tile,
  moving = wv tile), so phase 2 needs no PE transposes and no identity input.
- Causal masking is applied on-device with gpsimd.affine_select on the
  exp'd scores (fill 0), so no mask tensor is shipped or DMA'd.
- Weights for q&k of each head are packed adjacently so weight DMAs have
  512B-contiguous runs (256-col tiles), avoiding the small-elem DMA penalty.
- q/k head dims are de-interleaved (evens then odds) on the host so RoPE is
  full-tile DVE ops on partition halves; consistent between q and k.
- Scores are computed transposed ([tk, tq]); softmax is unnormalized exp
  with the denominator from a ones-vector matmul, divided into the output.
  Phase 2 is software-pipelined: scores for block i+2 are issued before the
  P*V / denominator matmuls of block i, hiding the ACT/Pool latency.
"""

import sys
sys.path.insert(0, '/opt/trn_rl_repo')
sys.path.insert(0, '/opt/trn_rl_repo/concourse')

import numpy as np

S = 2048
D = 4096
HD = 128
FSH = 1024            # features per core (8 heads)
NHL = 8               # heads per core
KT = D // 128         # 32 contraction tiles for projections
TSTRIPS = S // 512    # 4 tq strips
NKT = S // 128        # 16 tk tiles
NEG_THRESH = -1.0e8
PDEPTH = 2            # phase-2 software pipeline depth

# packed input layout (f16 element offsets)
O_WQK = 0
O_WVT = O_WQK + D * 2 * FSH
O_WOT = O_WVT + D * FSH
O_XT = O_WOT + FSH * D
O_CS = O_XT + D * S
N_PACKED = O_CS + 128 * S

_cache = {}


def _build(classes):
    """Build + compile the per-core Bacc program. classes[j][s] in {0:skip,1:full,2:partial}."""
    import concourse.bacc as bacc
    import concourse.mybir as mybir
    import concourse.tile as tile

    f32 = mybir.dt.float32
    f16 = mybir.dt.float16
    EXP = mybir.ActivationFunctionType.Exp
    GE = mybir.AluOpType.is_ge

    nc = bacc.Bacc("TRN2", target_bir_lowering=False, debug=False)

    packed = nc.dram_tensor("packed", [N_PACKED], f16, kind="ExternalInput").ap()
    wqk_d = packed[O_WQK:O_WVT].rearrange("(d f) -> d f", d=D)
    wvt_d = packed[O_WVT:O_WOT].rearrange("(d f) -> d f", d=D)
    wot_d = packed[O_WOT:O_XT].rearrange("(f d) -> f d", f=FSH)
    xt_d = packed[O_XT:O_CS].rearrange("(d s) -> d s", d=D)
    cs_d = packed[O_CS:N_PACKED].rearrange("(p s) -> p s", p=128)
    out_d = nc.dram_tensor("out", [S, D], f16, kind="ExternalOutput").ap()

    with tile.TileContext(nc) as tc, \
         nc.allow_low_precision(reason="f16 datapath, f32 psum accumulation"):
        with tc.tile_pool(name="pers", bufs=1) as pers:
            # persistent SBUF tensors (per-partition: 32+32+32 KB)
            qh = [pers.tile([128, S], f16, name=f"qh{h}") for h in range(NHL)]
            kh = [pers.tile([128, S], f16, name=f"kh{h}") for h in range(NHL)]
            vT = [pers.tile([128, FSH], f16, name=f"vT{j}") for j in range(NKT)]
            ones_sb = pers.tile([128, 1], f16, name="ones_sb")
            nc.vector.memset(ones_sb, 1.0)

            # ---------------- Phase V: V^T = (X @ Wv^T)^T -------------------
            with tc.tile_pool(name="pvw", bufs=1) as pvw, \
                 tc.tile_pool(name="pvx", bufs=2) as pvx, \
                 tc.tile_pool(name="psv", bufs=2, space="PSUM") as psv:
                wvh = []
                for half in range(2):
                    wt = pvw.tile([128, KT, 512], f16, name=f"wvh{half}")
                    wsrc = wvt_d[:, half * 512:(half + 1) * 512].rearrange(
                        "(k p) f -> p k f", p=128)
                    for q in range(4):
                        nc.sync.dma_start(out=wt[:, q * 8:(q + 1) * 8, :],
                                          in_=wsrc[:, q * 8:(q + 1) * 8, :])
                    wvh.append(wt)
                for jp in range(NKT // 2):
                    xc = pvx.tile([128, KT, 256], f16, name="xc")
                    nc.sync.dma_start(
                        out=xc,
                        in_=xt_d[:, jp * 256:(jp + 1) * 256].rearrange(
                            "(k p) s -> p k s", p=128))
                    for sub in range(2):
                        j = jp * 2 + sub
                        for half in range(2):
                            ps = psv.tile([128, 512], f32, name="psv")
                            for k in range(KT):
                                nc.tensor.matmul(ps, xc[:, k, sub * 128:(sub + 1) * 128],
                                                 wvh[half][:, k, :],
                                                 start=(k == 0), stop=(k == KT - 1))
                            nc.scalar.copy(vT[j][:, half * 512:(half + 1) * 512], ps)

            # ---------------- Phase QK: projections + RoPE ------------------
            with tc.tile_pool(name="pqx", bufs=2) as pqx, \
                 tc.tile_pool(name="pqw", bufs=2) as pqw, \
                 tc.tile_pool(name="pcs", bufs=1) as pcs, \
                 tc.tile_pool(name="prt", bufs=4) as prt, \
                 tc.tile_pool(name="psq", bufs=3, space="PSUM") as psq:
                cs_sb = pcs.tile([128, S], f16, name="cs_sb")
                nc.sync.dma_start(out=cs_sb, in_=cs_d)
                cos_sb, sin_sb = cs_sb[0:64], cs_sb[64:128]

                dests = [qh, kh]
                jobs = [(strip, i) for strip in range(TSTRIPS)
                        for i in range(NHL)]

                def load_w(strip, i):
                    # q|k of head i packed adjacently: 256 cols, 512B runs
                    wt = pqw.tile([128, KT, 256], f16, name="wt")
                    nc.sync.dma_start(
                        out=wt,
                        in_=wqk_d[:, i * 256:(i + 1) * 256].rearrange(
                            "(k p) f -> p k f", p=128))
                    return wt

                def load_x(strip):
                    t0 = strip * 512
                    xk = pqx.tile([128, KT, 512], f16, name="xk")
                    xsrc = xt_d[:, t0:t0 + 512].rearrange(
                        "(k p) s -> p k s", p=128)
                    for q in range(2):
                        nc.sync.dma_start(out=xk[:, q * 16:(q + 1) * 16, :],
                                          in_=xsrc[:, q * 16:(q + 1) * 16, :])
                    return xk

                xk = load_x(0)
                wt_next = load_w(*jobs[0])
                for idx, (strip, i) in enumerate(jobs):
                    t0 = strip * 512
                    if idx > 0 and idx % NHL == 0:
                        xk = load_x(strip)
                    csl = slice(t0, t0 + 512)
                    wt = wt_next
                    if idx + 1 < len(jobs):
                        wt_next = load_w(*jobs[idx + 1])
                    for proj in range(2):
                        wsl = slice(proj * 128, (proj + 1) * 128)
                        ps = psq.tile([128, 512], f32, name="psq")
                        for k in range(KT):
                            nc.tensor.matmul(ps, wt[:, k, wsl], xk[:, k, :],
                                             start=(k == 0), stop=(k == KT - 1))
                        dst = dests[proj][i]
                        # RoPE: partitions 0:64 = even dims (xr), 64:128 = odd (xi)
                        m1 = prt.tile([64, 512], f16, name="m1")
                        m2 = prt.tile([64, 512], f16, name="m2")
                        nc.vector.tensor_mul(m1, ps[0:64], cos_sb[:, csl])
                        nc.vector.tensor_mul(m2, ps[64:128], sin_sb[:, csl])
                        nc.vector.tensor_sub(dst[0:64, t0:t0 + 512], m1, m2)
                        m3 = prt.tile([64, 512], f16, name="m1")
                        m4 = prt.tile([64, 512], f16, name="m2")
                        nc.vector.tensor_mul(m3, ps[0:64], sin_sb[:, csl])
                        nc.vector.tensor_mul(m4, ps[64:128], cos_sb[:, csl])
                        nc.vector.tensor_add(dst[64:128, t0:t0 + 512], m3, m4)

            # ---------------- Phase 2: attention per head -------------------
            with tc.tile_pool(name="patt", bufs=1) as patt:
              att_sb = [patt.tile([128, S], f16, name=f"attT{h}") for h in range(NHL)]
              with tc.tile_pool(name="p2e", bufs=8) as p2e, \
                   tc.tile_pool(name="p2r", bufs=2) as p2r, \
                   tc.tile_pool(name="p2b", bufs=2) as p2b, \
                   tc.tile_pool(name="ps2s", bufs=4, space="PSUM") as ps2s, \
                   tc.tile_pool(name="ps2a", bufs=2, space="PSUM") as ps2a, \
                   tc.tile_pool(name="ps2d", bufs=2, space="PSUM") as ps2d:
                  # software pipeline: scores for block n+PDEPTH issue before
                  # the PV/denominator matmuls of block n
                  flat = []
                  for h in range(NHL):
                      for s in range(TSTRIPS):
                          act = [j for j in range(NKT) if classes[j][s] != 0]
                          flat.extend((h, s, act, idx, j)
                                      for idx, j in enumerate(act))
                  Es, AD = {}, {}

                  def emit_scores(n):
                      h, s, act, idx, j = flat[n]
                      if idx == 0:
                          AD[(h, s)] = (ps2a.tile([128, 512], f32, name="A"),
                                        ps2d.tile([1, 512], f32, name="Dn"))
                      sps = ps2s.tile([128, 512], f32, name="sps")
                      nc.tensor.matmul(sps, kh[h][:, j * 128:(j + 1) * 128],
                                       qh[h][:, s * 512:(s + 1) * 512],
                                       start=True, stop=True)
                      E = p2e.tile([128, 512], f16, name="E")
                      nc.scalar.activation(E, sps, EXP)
                      if classes[j][s] == 2:
                          # zero the not-yet-valid (q < k) entries
                          nc.gpsimd.affine_select(
                              E, E, pattern=[[1, 512]], compare_op=GE,
                              fill=0.0, base=s * 512 - j * 128,
                              channel_multiplier=-1)
                      Es[n] = E

                  def emit_av(n):
                      h, s, act, idx, j = flat[n]
                      A, Dn = AD[(h, s)]
                      E = Es.pop(n)
                      first, last = (idx == 0), (idx == len(act) - 1)
                      nc.tensor.matmul(A, vT[j][:, h * 128:(h + 1) * 128], E,
                                       start=first, stop=last)
                      nc.tensor.matmul(Dn, ones_sb, E, start=first, stop=last)
                      if last:
                          rec = p2r.tile([1, 512], f32, name="rec")
                          nc.vector.reciprocal(rec, Dn[0:1, :])
                          bsb = p2b.tile([128, 512], f32, name="bsb")
                          nc.gpsimd.partition_broadcast(bsb, rec, 128)
                          nc.vector.tensor_mul(
                              att_sb[h][:, s * 512:(s + 1) * 512], A, bsb)
                          del AD[(h, s)]

                  for n in range(len(flat)):
                      emit_scores(n)
                      if n >= PDEPTH:
                          emit_av(n - PDEPTH)
                  for n in range(len(flat) - PDEPTH, len(flat)):
                      emit_av(n)

              # ---------------- Phase 3: output projection ------------------
              with tc.tile_pool(name="p3w", bufs=2) as p3w, \
                   tc.tile_pool(name="p3o", bufs=4) as p3o, \
                   tc.tile_pool(name="ps3", bufs=4, space="PSUM") as ps3:
                  def load_w3(c):
                      wt = p3w.tile([128, NHL, 512], f16, name="w3")
                      nc.sync.dma_start(
                          out=wt,
                          in_=wot_d[:, c * 512:(c + 1) * 512].rearrange(
                              "(k p) f -> p k f", p=128))
                      return wt

                  wt_next3 = load_w3(0)
                  for c in range(8):        # dout chunks of 512
                      wt = wt_next3
                      if c + 1 < 8:
                          wt_next3 = load_w3(c + 1)
                      for m in range(NKT):  # t tiles of 128
                          ps = ps3.tile([128, 512], f32, name="ps3")
                          for k in range(NHL):
                              nc.tensor.matmul(ps, att_sb[k][:, m * 128:(m + 1) * 128],
                                               wt[:, k, :],
                                               start=(k == 0), stop=(k == NHL - 1))
                          ot = p3o.tile([128, 512], f16, name="o3")
                          nc.scalar.copy(ot, ps)
                          nc.gpsimd.dma_start(
                              out=out_d[m * 128:(m + 1) * 128, c * 512:(c + 1) * 512],
                              in_=ot)

    nc.compile()
    return nc


def _host_prep(x, wq, wk, wv, wo, freqs_cos, freqs_sin, mask):
    """Build per-core input maps + mask block classes."""
    x = np.asarray(x, np.float32)
    wq = np.asarray(wq, np.float32)
    wk = np.asarray(wk, np.float32)
    wv = np.asarray(wv, np.float32)
    wo = np.asarray(wo, np.float32)
    mask2 = np.asarray(mask, np.float32).reshape(S, S)

    perm = np.concatenate(
        [hl * 128 + np.concatenate([np.arange(0, 128, 2), np.arange(1, 128, 2)])
         for hl in range(NHL)])
    cs = np.empty((128, S), np.float16)
    cs[0:64] = np.asarray(freqs_cos, np.float32).T
    cs[64:128] = np.asarray(freqs_sin, np.float32).T

    classes = [[0] * TSTRIPS for _ in range(NKT)]
    for j in range(NKT):
        for s in range(TSTRIPS):
            blk = mask2.T[j * 128:(j + 1) * 128, s * 512:(s + 1) * 512]
            if (blk <= NEG_THRESH).all():
                classes[j][s] = 0
            elif (blk == 0.0).all():
                classes[j][s] = 1
            else:
                classes[j][s] = 2
                # the kernel masks partial blocks with an on-device causal
                # staircase; verify the given mask block matches it
                r = np.arange(128)[:, None]
                c = np.arange(512)[None, :]
                valid = (s * 512 + c) >= (j * 128 + r)
                assert ((blk == 0.0) == valid).all() and \
                       ((blk <= NEG_THRESH) == ~valid).all(), \
                    f"mask block ({j},{s}) is not the causal staircase"

    xts = [np.ascontiguousarray(x[b].T).astype(np.float16) for b in range(2)]
    in_maps = []
    for core in range(8):
        b, tp = core // 4, core % 4
        sl = slice(tp * FSH, (tp + 1) * FSH)
        wq_t = np.ascontiguousarray(
            (wq[sl][perm] * np.float32(1.0 / np.sqrt(HD))).T).astype(np.float16)
        wk_t = np.ascontiguousarray(wk[sl][perm].T).astype(np.float16)
        wqk = np.empty((D, 2 * FSH), np.float16)
        for h in range(NHL):
            wqk[:, h * 256:h * 256 + 128] = wq_t[:, h * 128:(h + 1) * 128]
            wqk[:, h * 256 + 128:(h + 1) * 256] = wk_t[:, h * 128:(h + 1) * 128]
        pk = np.empty(N_PACKED, np.float16)
        pk[O_WQK:O_WVT] = wqk.ravel()
        pk[O_WVT:O_WOT] = np.ascontiguousarray(wv[sl].T).astype(np.float16).ravel()
        pk[O_WOT:O_XT] = np.ascontiguousarray(wo[:, sl].T).astype(np.float16).ravel()
        pk[O_XT:O_CS] = xts[b].ravel()
        pk[O_CS:N_PACKED] = cs.ravel()
        in_maps.append({"packed": pk})
    return in_maps, classes


def kernel(x, wq, wk, wv, wo, freqs_cos, freqs_sin, mask, start_pos=0,
           _trace=False):
    from concourse import bass_utils
    in_maps, classes = _host_prep(x, wq, wk, wv, wo, freqs_cos, freqs_sin, mask)
    key = str(classes)
    if key not in _cache:
        _cache[key] = _build(classes)
    nc = _cache[key]
    res = bass_utils.run_bass_kernel_spmd(nc, in_maps, core_ids=list(range(8)),
                                          trace=_trace)
    out = np.zeros((2, S, D), np.float32)
    for core in range(8):
        out[core // 4] += res.results[core]["out"].astype(np.float32)
    kernel.last_result = res
    return out


if __name__ == "__main__":
    # compile-only smoke test
    classes = [[2 if j * 128 <= s * 512 + 511 and j * 128 + 127 > s * 512 else
                (1 if j * 128 + 127 <= s * 512 else 0)
                for s in range(TSTRIPS)] for j in range(NKT)]
    import time
    t0 = time.time()
    nc = _build(classes)
    print(f"build+bacc-compile: {time.time()-t0:.1f}s")
    try:
        from concourse.timeline_sim import TimelineSim
        est = TimelineSim(nc, trace=False).simulate()
        print(f"TimelineSim per-core exec estimate: {est:.0f} ns")
    except Exception as e:
        print("TimelineSim unavailable:", e)
    if len(sys.argv) > 1 and sys.argv[1] == "neff":
        import tempfile
        from concourse import bass_utils
        t0 = time.time()
        with tempfile.TemporaryDirectory() as td:
            bass_utils.compile_bass_kernel(nc, td)
            print(f"walrus: {time.time()-t0:.1f}s COMPILED OK")


# revision 24
# speedup vs baseline: 54.6353x; 1.1686x over previous
"""Multi-head causal attention (B=2, S=2048, D=4096, H=32, hd=128) on 8 trn2 cores.

Sharding: DP over batch (2) x TP over heads (4 groups of 8 heads).
Core c: batch b = c//4, head-group tp = c%4.
Each core computes a partial output [2048, 4096] (wo row-sharded); host sums
the 4 partials per batch.

v3 design (f16 datapath, SBUF-resident intermediates, single packed input):
- All matmul operands are float16 (PE runs f16 at 1 cycle/row, same as f32r,
  but DMA bytes and SBUF footprint halve; PSUM accumulation stays f32).
- All per-core inputs live in ONE packed DRAM tensor (the axon dispatch path
  costs ~70us/exec per IO buffer, independent of bytes) viewed through
  sliced/rearranged APs.
- q/k/v^T/att live entirely in SBUF between phases -- no DRAM spill.
- V^T is produced directly by the projection matmul (stationary = x tile,
  moving = wv tile), so phase 2 needs no PE transposes and no identity input.
- Causal masking is applied on-device with gpsimd.affine_select on the
  exp'd scores (fill 0), so no mask tensor is shipped or DMA'd.
- Weights for q&k of each head are packed adjacently so weight DMAs have
  512B-contiguous runs (256-col tiles), avoiding the small-elem DMA penalty.
- q/k head dims are de-interleaved (evens then odds) on the host so RoPE is
  full-tile DVE ops on partition halves; consistent between q and k.
- Scores are computed transposed ([tk, tq]); softmax is unnormalized exp
  with the denominator from a ones-vector matmul, divided into the output.
  Phase 2 is software-pipelined: scores for block i+PDEPTH are issued before
  the P*V / denominator matmuls of block i, hiding the ACT/Pool latency.
- The wo output projection is fused into the attention pipeline (strip-outer
  order): as each 512-query strip finishes normalizing across all heads, its
  wo chains are queued and drained one per attention item, so the PE stream
  never drains between attention and projection.
"""

import sys
sys.path.insert(0, '/opt/trn_rl_repo')
sys.path.insert(0, '/opt/trn_rl_repo/concourse')

import numpy as np

S = 2048
D = 4096
HD = 128
FSH = 1024            # features per core (8 heads)
NHL = 8               # heads per core
KT = D // 128         # 32 contraction tiles for projections
TSTRIPS = S // 512    # 4 tq strips
NKT = S // 128        # 16 tk tiles
NEG_THRESH = -1.0e8
PDEPTH = 2            # phase-2 software pipeline depth

# packed input layout (f16 element offsets)
O_WQK = 0
O_WVT = O_WQK + D * 2 * FSH
O_WOT = O_WVT + D * FSH
O_XT = O_WOT + FSH * D
O_CS = O_XT + D * S
N_PACKED = O_CS + 128 * S

_cache = {}


def _build(classes):
    """Build + compile the per-core Bacc program. classes[j][s] in {0:skip,1:full,2:partial}."""
    import concourse.bacc as bacc
    import concourse.mybir as mybir
    import concourse.tile as tile

    f32 = mybir.dt.float32
    f16 = mybir.dt.float16
    EXP = mybir.ActivationFunctionType.Exp
    GE = mybir.AluOpType.is_ge

    nc = bacc.Bacc("TRN2", target_bir_lowering=False, debug=False)

    packed = nc.dram_tensor("packed", [N_PACKED], f16, kind="ExternalInput").ap()
    wqk_d = packed[O_WQK:O_WVT].rearrange("(d f) -> d f", d=D)
    wvt_d = packed[O_WVT:O_WOT].rearrange("(d f) -> d f", d=D)
    wot_d = packed[O_WOT:O_XT].rearrange("(f d) -> f d", f=FSH)
    xt_d = packed[O_XT:O_CS].rearrange("(d s) -> d s", d=D)
    cs_d = packed[O_CS:N_PACKED].rearrange("(p s) -> p s", p=128)
    out_d = nc.dram_tensor("out", [S, D], f16, kind="ExternalOutput").ap()

    with tile.TileContext(nc) as tc, \
         nc.allow_low_precision(reason="f16 datapath, f32 psum accumulation"):
        with tc.tile_pool(name="pers", bufs=1) as pers:
            # persistent SBUF tensors (per-partition: 32+32+32 KB)
            qh = [pers.tile([128, S], f16, name=f"qh{h}") for h in range(NHL)]
            kh = [pers.tile([128, S], f16, name=f"kh{h}") for h in range(NHL)]
            vT = [pers.tile([128, FSH], f16, name=f"vT{j}") for j in range(NKT)]
            ones_sb = pers.tile([128, 1], f16, name="ones_sb")
            nc.vector.memset(ones_sb, 1.0)

            # ---------------- Phase V: V^T = (X @ Wv^T)^T -------------------
            with tc.tile_pool(name="pvw", bufs=1) as pvw, \
                 tc.tile_pool(name="pvx", bufs=2) as pvx, \
                 tc.tile_pool(name="psv", bufs=2, space="PSUM") as psv:
                wvh = []
                for half in range(2):
                    wt = pvw.tile([128, KT, 512], f16, name=f"wvh{half}")
                    wsrc = wvt_d[:, half * 512:(half + 1) * 512].rearrange(
                        "(k p) f -> p k f", p=128)
                    for q in range(4):
                        nc.sync.dma_start(out=wt[:, q * 8:(q + 1) * 8, :],
                                          in_=wsrc[:, q * 8:(q + 1) * 8, :])
                    wvh.append(wt)
                for jp in range(NKT // 2):
                    xc = pvx.tile([128, KT, 256], f16, name="xc")
                    xcs = xt_d[:, jp * 256:(jp + 1) * 256].rearrange(
                        "(k p) s -> p k s", p=128)
                    for q in range(2):
                        nc.sync.dma_start(out=xc[:, q * 16:(q + 1) * 16, :],
                                          in_=xcs[:, q * 16:(q + 1) * 16, :])
                    for sub in range(2):
                        j = jp * 2 + sub
                        for half in range(2):
                            ps = psv.tile([128, 512], f32, name="psv")
                            for k in range(KT):
                                nc.tensor.matmul(ps, xc[:, k, sub * 128:(sub + 1) * 128],
                                                 wvh[half][:, k, :],
                                                 start=(k == 0), stop=(k == KT - 1))
                            nc.scalar.copy(vT[j][:, half * 512:(half + 1) * 512], ps)

            # ---------------- Phase QK: projections + RoPE ------------------
            with tc.tile_pool(name="pqx", bufs=2) as pqx, \
                 tc.tile_pool(name="pqw", bufs=2) as pqw, \
                 tc.tile_pool(name="pcs", bufs=1) as pcs, \
                 tc.tile_pool(name="prt", bufs=4) as prt, \
                 tc.tile_pool(name="psq", bufs=3, space="PSUM") as psq:
                cs_sb = pcs.tile([128, S], f16, name="cs_sb")
                nc.sync.dma_start(out=cs_sb, in_=cs_d)
                cos_sb, sin_sb = cs_sb[0:64], cs_sb[64:128]

                dests = [qh, kh]
                jobs = [(strip, i) for strip in range(TSTRIPS)
                        for i in range(NHL)]

                def load_w(strip, i):
                    # q|k of head i packed adjacently: 256 cols, 512B runs
                    wt = pqw.tile([128, KT, 256], f16, name="wt")
                    nc.sync.dma_start(
                        out=wt,
                        in_=wqk_d[:, i * 256:(i + 1) * 256].rearrange(
                            "(k p) f -> p k f", p=128))
                    return wt

                def load_x(strip):
                    t0 = strip * 512
                    xk = pqx.tile([128, KT, 512], f16, name="xk")
                    xsrc = xt_d[:, t0:t0 + 512].rearrange(
                        "(k p) s -> p k s", p=128)
                    for q in range(2):
                        nc.sync.dma_start(out=xk[:, q * 16:(q + 1) * 16, :],
                                          in_=xsrc[:, q * 16:(q + 1) * 16, :])
                    return xk

                xk = load_x(0)
                wt_next = load_w(*jobs[0])
                for idx, (strip, i) in enumerate(jobs):
                    t0 = strip * 512
                    if idx > 0 and idx % NHL == 0:
                        xk = load_x(strip)
                    csl = slice(t0, t0 + 512)
                    wt = wt_next
                    if idx + 1 < len(jobs):
                        wt_next = load_w(*jobs[idx + 1])
                    for proj in range(2):
                        wsl = slice(proj * 128, (proj + 1) * 128)
                        ps = psq.tile([128, 512], f32, name="psq")
                        for k in range(KT):
                            nc.tensor.matmul(ps, wt[:, k, wsl], xk[:, k, :],
                                             start=(k == 0), stop=(k == KT - 1))
                        dst = dests[proj][i]
                        # RoPE: partitions 0:64 = even dims (xr), 64:128 = odd (xi)
                        m1 = prt.tile([64, 512], f16, name="m1")
                        m2 = prt.tile([64, 512], f16, name="m2")
                        nc.vector.tensor_mul(m1, ps[0:64], cos_sb[:, csl])
                        nc.vector.tensor_mul(m2, ps[64:128], sin_sb[:, csl])
                        nc.vector.tensor_sub(dst[0:64, t0:t0 + 512], m1, m2)
                        m3 = prt.tile([64, 512], f16, name="m1")
                        m4 = prt.tile([64, 512], f16, name="m2")
                        nc.vector.tensor_mul(m3, ps[0:64], sin_sb[:, csl])
                        nc.vector.tensor_mul(m4, ps[64:128], cos_sb[:, csl])
                        nc.vector.tensor_add(dst[64:128, t0:t0 + 512], m3, m4)

            # ------- Phase 2+3 fused: attention + output projection ---------
            # s-outer order; after the last head of strip s is normalized,
            # the wo m-tiles of that strip are emitted into the same PE
            # stream (wo chains share the scores psum pool), so PE never
            # drains between attention and projection.
            with tc.tile_pool(name="patt", bufs=1) as patt, \
                 tc.tile_pool(name="p3w", bufs=1) as p3w, \
                 tc.tile_pool(name="p2e", bufs=8) as p2e, \
                 tc.tile_pool(name="p2r", bufs=1) as p2r, \
                 tc.tile_pool(name="p2ac", bufs=2) as p2ac, \
                 tc.tile_pool(name="p2b", bufs=1) as p2b, \
                 tc.tile_pool(name="p3o", bufs=4) as p3o, \
                 tc.tile_pool(name="ps2s", bufs=3, space="PSUM") as ps2s, \
                 tc.tile_pool(name="ps2a", bufs=2, space="PSUM") as ps2a, \
                 tc.tile_pool(name="ps2d", bufs=1, space="PSUM") as ps2d, \
                 tc.tile_pool(name="ps3w", bufs=2, space="PSUM") as ps3w:
                  # att tiles rotate over 2 strip slots: written in strip s,
                  # consumed by the wo chains drained during strip s+1
                  att_sb = [patt.tile([128, 1024], f16, name=f"attT{h}")
                            for h in range(NHL)]
                  w3 = []
                  for c in range(8):
                      wt = p3w.tile([128, NHL, 512], f16, name=f"w3c{c}")
                      nc.sync.dma_start(
                          out=wt,
                          in_=wot_d[:, c * 512:(c + 1) * 512].rearrange(
                              "(k p) f -> p k f", p=128))
                      w3.append(wt)

                  flat = []
                  for s in range(TSTRIPS):
                      for h in range(NHL):
                          act = [j for j in range(NKT) if classes[j][s] != 0]
                          flat.extend((h, s, act, idx, j)
                                      for idx, j in enumerate(act))
                  Es, AD = {}, {}

                  def emit_scores(n):
                      h, s, act, idx, j = flat[n]
                      if idx == 0:
                          AD[(h, s)] = (ps2a.tile([128, 512], f32, name="A"),
                                        p2ac.tile([128, 512], f16, name="Eac"))
                      sps = ps2s.tile([128, 512], f32, name="sps")
                      nc.tensor.matmul(sps, kh[h][:, j * 128:(j + 1) * 128],
                                       qh[h][:, s * 512:(s + 1) * 512],
                                       start=True, stop=True)
                      E = p2e.tile([128, 512], f16, name="E")
                      nc.scalar.activation(E, sps, EXP)
                      if classes[j][s] == 2:
                          # zero the not-yet-valid (q < k) entries
                          nc.gpsimd.affine_select(
                              E, E, pattern=[[1, 512]], compare_op=GE,
                              fill=0.0, base=s * 512 - j * 128,
                              channel_multiplier=-1)
                      Eac = AD[(h, s)][1]
                      if idx == 0:
                          nc.vector.tensor_copy(Eac, E)
                      else:
                          nc.vector.tensor_add(Eac, Eac, E)
                      Es[n] = E

                  wo_pending = []

                  def emit_wo_one():
                      m, c = wo_pending.pop(0)
                      ps = ps3w.tile([128, 512], f32, name="ps3")
                      mo = ((m // 4) % 2) * 512 + (m % 4) * 128
                      for k in range(NHL):
                          nc.tensor.matmul(
                              ps, att_sb[k][:, mo:mo + 128],
                              w3[c][:, k, :],
                              start=(k == 0), stop=(k == NHL - 1))
                      ot = p3o.tile([128, 512], f16, name="o3")
                      nc.scalar.copy(ot, ps)
                      nc.gpsimd.dma_start(
                          out=out_d[m * 128:(m + 1) * 128,
                                    c * 512:(c + 1) * 512],
                          in_=ot)

                  def emit_wo(s):
                      # queue the strip's output-projection chains; they are
                      # drained one per attention item to spread PE work
                      wo_pending.extend((m, c) for m in range(4 * s, 4 * s + 4)
                                        for c in range(8))

                  def emit_av(n):
                      h, s, act, idx, j = flat[n]
                      A, Eac = AD[(h, s)]
                      E = Es.pop(n)
                      first, last = (idx == 0), (idx == len(act) - 1)
                      nc.tensor.matmul(A, vT[j][:, h * 128:(h + 1) * 128], E,
                                       start=first, stop=last)
                      if last:
                          # one partition-reduction matmul per (h,s) over the
                          # DVE-accumulated exp sum instead of one per block
                          Dn = ps2d.tile([1, 512], f32, name="Dn")
                          nc.tensor.matmul(Dn, ones_sb, Eac,
                                           start=True, stop=True)
                          rec = p2r.tile([1, 512], f32, name="rec")
                          nc.vector.reciprocal(rec, Dn[0:1, :])
                          bsb = p2b.tile([128, 512], f32, name="bsb")
                          nc.gpsimd.partition_broadcast(bsb, rec, 128)
                          sl = (s % 2) * 512
                          nc.vector.tensor_mul(
                              att_sb[h][:, sl:sl + 512], A, bsb)
                          del AD[(h, s)]
                          if h == NHL - 1:
                              emit_wo(s)

                  for n in range(len(flat)):
                      emit_scores(n)
                      if n >= PDEPTH:
                          emit_av(n - PDEPTH)
                      if wo_pending:
                          emit_wo_one()
                  for n in range(len(flat) - PDEPTH, len(flat)):
                      emit_av(n)
                  while wo_pending:
                      emit_wo_one()

    nc.compile()
    return nc


def _host_prep(x, wq, wk, wv, wo, freqs_cos, freqs_sin, mask):
    """Build per-core input maps + mask block classes."""
    x = np.asarray(x, np.float32)
    wq = np.asarray(wq, np.float32)
    wk = np.asarray(wk, np.float32)
    wv = np.asarray(wv, np.float32)
    wo = np.asarray(wo, np.float32)
    mask2 = np.asarray(mask, np.float32).reshape(S, S)

    perm = np.concatenate(
        [hl * 128 + np.concatenate([np.arange(0, 128, 2), np.arange(1, 128, 2)])
         for hl in range(NHL)])
    cs = np.empty((128, S), np.float16)
    cs[0:64] = np.asarray(freqs_cos, np.float32).T
    cs[64:128] = np.asarray(freqs_sin, np.float32).T

    classes = [[0] * TSTRIPS for _ in range(NKT)]
    for j in range(NKT):
        for s in range(TSTRIPS):
            blk = mask2.T[j * 128:(j + 1) * 128, s * 512:(s + 1) * 512]
            if (blk <= NEG_THRESH).all():
                classes[j][s] = 0
            elif (blk == 0.0).all():
                classes[j][s] = 1
            else:
                classes[j][s] = 2
                # the kernel masks partial blocks with an on-device causal
                # staircase; verify the given mask block matches it
                r = np.arange(128)[:, None]
                c = np.arange(512)[None, :]
                valid = (s * 512 + c) >= (j * 128 + r)
                assert ((blk == 0.0) == valid).all() and \
                       ((blk <= NEG_THRESH) == ~valid).all(), \
                    f"mask block ({j},{s}) is not the causal staircase"

    xts = [np.ascontiguousarray(x[b].T).astype(np.float16) for b in range(2)]
    in_maps = []
    for core in range(8):
        b, tp = core // 4, core % 4
        sl = slice(tp * FSH, (tp + 1) * FSH)
        wq_t = np.ascontiguousarray(
            (wq[sl][perm] * np.float32(1.0 / np.sqrt(HD))).T).astype(np.float16)
        wk_t = np.ascontiguousarray(wk[sl][perm].T).astype(np.float16)
        wqk = np.empty((D, 2 * FSH), np.float16)
        for h in range(NHL):
            wqk[:, h * 256:h * 256 + 128] = wq_t[:, h * 128:(h + 1) * 128]
            wqk[:, h * 256 + 128:(h + 1) * 256] = wk_t[:, h * 128:(h + 1) * 128]
        pk = np.empty(N_PACKED, np.float16)
        pk[O_WQK:O_WVT] = wqk.ravel()
        pk[O_WVT:O_WOT] = np.ascontiguousarray(wv[sl].T).astype(np.float16).ravel()
        pk[O_WOT:O_XT] = np.ascontiguousarray(wo[:, sl].T).astype(np.float16).ravel()
        pk[O_XT:O_CS] = xts[b].ravel()
        pk[O_CS:N_PACKED] = cs.ravel()
        in_maps.append({"packed": pk})
    return in_maps, classes


def kernel(x, wq, wk, wv, wo, freqs_cos, freqs_sin, mask, start_pos=0,
           _trace=False):
    from concourse import bass_utils
    in_maps, classes = _host_prep(x, wq, wk, wv, wo, freqs_cos, freqs_sin, mask)
    key = str(classes)
    if key not in _cache:
        _cache[key] = _build(classes)
    nc = _cache[key]
    res = bass_utils.run_bass_kernel_spmd(nc, in_maps, core_ids=list(range(8)),
                                          trace=_trace)
    out = np.zeros((2, S, D), np.float32)
    for core in range(8):
        out[core // 4] += res.results[core]["out"].astype(np.float32)
    kernel.last_result = res
    return out


if __name__ == "__main__":
    # compile-only smoke test
    classes = [[2 if j * 128 <= s * 512 + 511 and j * 128 + 127 > s * 512 else
                (1 if j * 128 + 127 <= s * 512 else 0)
                for s in range(TSTRIPS)] for j in range(NKT)]
    import time
    t0 = time.time()
    nc = _build(classes)
    print(f"build+bacc-compile: {time.time()-t0:.1f}s")
    try:
        from concourse.timeline_sim import TimelineSim
        est = TimelineSim(nc, trace=False).simulate()
        print(f"TimelineSim per-core exec estimate: {est:.0f} ns")
    except Exception as e:
        print("TimelineSim unavailable:", e)
    if len(sys.argv) > 1 and sys.argv[1] == "neff":
        import tempfile
        from concourse import bass_utils
        t0 = time.time()
        with tempfile.TemporaryDirectory() as td:
            bass_utils.compile_bass_kernel(nc, td)
            print(f"walrus: {time.time()-t0:.1f}s COMPILED OK")
